# revision 16
# baseline (speedup 1.0000x reference)
"""BilinearPooling kernel for TRN2 (8 NeuronCores, pure data parallel).

Reference math: out[b, k] = mean_j(conv1[b, j]) * conv2[b, k], with
conv1/conv2 flattened to [B, 50176] from [256, 14, 14, 256].

Sharding: batch dim B=256 split across 8 cores -> 32 samples/core.
Per-core layout: the [32, 50176] slice is viewed as [128, 12544] so sample b
occupies partitions 4b..4b+3.  A free-axis reduce gives per-partition partial
sums; one fp32 matmul against a block-diagonal (1/J) matrix sums each group of
4 partitions and broadcasts the per-sample mean back to its 4 partitions.
conv2 streams through SBUF with a per-partition scalar multiply.

Raw Bass (no Tile): the DGE DMA instruction supports at most one attached
sync-wait, so all waits are standalone engine wait_ge instructions and every
dma_start carries none.  Engine roles: SP streams the c1/c2 loads (HWDGE),
DVE does reduces + multiplies (reading the scale vector straight from PSUM),
PE does the tiny block-diag matmul, ACT loads the block-diag constant and
issues the stores (HWDGE).
"""

from contextlib import ExitStack

import numpy as np

import concourse.bass as bass
import concourse.mybir as mybir
from concourse.bass_utils import run_bass_kernel_spmd

B = 256          # full batch
J = 50176        # flattened feature dim (14*14*256)
NCORES = 8
BPC = B // NCORES          # 32 samples per core
P = 128                    # SBUF partitions
RPS = P // BPC             # 4 partition-rows per sample
F = J // RPS               # 12544 free elems per partition
NCHUNK = 8
CHUNK = F // NCHUNK        # 3136 (-> [128, 3136] f32 tiles, 1.6 MB)

FP32 = mybir.dt.float32
AX = mybir.AxisListType.X

# Stashed by kernel() for test harnesses that want timing/trace info.
LAST_RESULTS = None


def _build_nc():
    nc = bass.Bass()
    c1 = nc.dram_tensor("conv1", [P, F], FP32, kind="ExternalInput")
    c2 = nc.dram_tensor("conv2", [P, F], FP32, kind="ExternalInput")
    bd = nc.dram_tensor("blockdiag", [P, P], FP32, kind="ExternalInput")
    out = nc.dram_tensor("out", [P, F], FP32, kind="ExternalOutput")

    with ExitStack() as ctx:
        ec = ctx.enter_context
        c1t = [ec(nc.sbuf_tensor(f"c1t{i}", [P, CHUNK], FP32)) for i in range(NCHUNK)]
        c2t = [ec(nc.sbuf_tensor(f"c2t{i}", [P, CHUNK], FP32)) for i in range(NCHUNK)]
        ot = [ec(nc.sbuf_tensor(f"ot{i}", [P, CHUNK], FP32)) for i in range(NCHUNK)]
        bdt = ec(nc.sbuf_tensor("bdt", [P, P], FP32))
        partials = ec(nc.sbuf_tensor("partials", [P, NCHUNK], FP32))
        sums = ec(nc.sbuf_tensor("sums", [P, 1], FP32))
        pscale = ec(nc.psum_tensor("pscale", [P, 1], FP32))

        bds = ec(nc.semaphore("bds"))
        c1s = [ec(nc.semaphore(f"c1s{i}")) for i in range(NCHUNK)]
        c2s = [ec(nc.semaphore(f"c2s{i}")) for i in range(NCHUNK)]
        c1red = ec(nc.semaphore("c1red"))
        red = ec(nc.semaphore("red"))
        mms = ec(nc.semaphore("mms"))
        muls = ec(nc.semaphore("muls"))
        sts = ec(nc.semaphore("sts"))

        block = ec(nc.Block(no_gpsimd_drain=True))

        @block.sync
        def _(sync):
            for i in range(NCHUNK):
                sync.dma_start(c1t[i][:], c1[:, bass.ts(i, CHUNK)]).then_inc(
                    c1s[i], 16
                )
            for i in range(NCHUNK):
                sync.dma_start(c2t[i][:], c2[:, bass.ts(i, CHUNK)]).then_inc(
                    c2s[i], 16
                )

        @block.vector
        def _(vector):
            for i in range(NCHUNK):
                vector.wait_ge(c1s[i], 16)
                nc.vector.reduce_sum(
                    partials[:, i : i + 1], c1t[i][:], axis=AX
                ).then_inc(c1red, 1)
            vector.wait_ge(c1red, NCHUNK)
            nc.vector.reduce_sum(sums[:], partials[:], axis=AX).then_inc(red, 1)
            vector.wait_ge(mms, 1)
            for i in range(NCHUNK):
                vector.wait_ge(c2s[i], 16)
                nc.vector.tensor_scalar_mul(
                    ot[i][:], c2t[i][:], pscale[:, 0:1]
                ).then_inc(muls, 1)

        @block.tensor
        def _(tensor):
            tensor.wait_ge(bds, 16)
            tensor.wait_ge(red, 1)
            nc.tensor.matmul(
                pscale[:], bdt[:], sums[:], start=True, stop=True
            ).then_inc(mms, 1)

        @block.scalar
        def _(scalar):
            scalar.dma_start(bdt[:], bd[:]).then_inc(bds, 16)
            # Defer stores until every load has landed: reads then run at full
            # fabric rate with no read/write turnaround on the HBM bus, and the
            # write phase gets a clean bus afterwards.
            scalar.wait_ge(c2s[NCHUNK - 1], 16)
            for i in range(NCHUNK):
                scalar.wait_ge(muls, i + 1)
                scalar.dma_start(out[:, bass.ts(i, CHUNK)], ot[i][:]).then_inc(
                    sts, 16
                )
            scalar.wait_ge(sts, 16 * NCHUNK)

    return nc


def kernel(conv1, conv2, _trace=False):
    global LAST_RESULTS
    conv1 = np.ascontiguousarray(np.asarray(conv1, dtype=np.float32))
    conv2 = np.ascontiguousarray(np.asarray(conv2, dtype=np.float32))
    c1 = conv1.reshape(B, J)
    c2 = conv2.reshape(B, J)

    # blockdiag[p, m] = 1/J if p//RPS == m//RPS else 0
    bd = (
        np.kron(np.eye(BPC, dtype=np.float32), np.ones((RPS, RPS), dtype=np.float32))
        / np.float32(J)
    ).astype(np.float32)

    in_maps = []
    for i in range(NCORES):
        sl = slice(i * BPC, (i + 1) * BPC)
        in_maps.append(
            {
                "conv1": c1[sl].reshape(P, F),
                "conv2": c2[sl].reshape(P, F),
                "blockdiag": bd,
            }
        )

    nc = _build_nc()
    res = run_bass_kernel_spmd(nc, in_maps, list(range(NCORES)), trace=bool(_trace))
    LAST_RESULTS = res
    out = np.concatenate(
        [res.results[i]["out"].reshape(BPC, J) for i in range(NCORES)], axis=0
    )
    return out


# revision 18
# speedup vs baseline: 1.0207x; 1.0207x over previous
"""BilinearPooling kernel for TRN2 (8 NeuronCores, pure data parallel).

Reference math: out[b, k] = mean_j(conv1[b, j]) * conv2[b, k], with
conv1/conv2 flattened to [B, 50176] from [256, 14, 14, 256].

Sharding: batch dim B=256 split across 8 cores -> 32 samples/core.
Per-core layout: the [32, 50176] slice is viewed as [128, 12544] so sample b
occupies partitions 4b..4b+3.  A free-axis reduce gives per-partition partial
sums; one fp32 matmul against a block-diagonal (1/J) matrix sums each group of
4 partitions and broadcasts the per-sample mean back to its 4 partitions.
conv2 streams through SBUF with a per-partition scalar multiply.

Raw Bass (no Tile): the DGE DMA instruction supports at most one attached
sync-wait, so all waits are standalone engine wait_ge instructions and every
dma_start carries none.  Engine roles: SP streams the c1/c2 loads (HWDGE),
DVE does reduces + multiplies (reading the scale vector straight from PSUM),
PE does the tiny block-diag matmul, ACT loads the block-diag constant and
issues the stores (HWDGE).
"""

from contextlib import ExitStack

import numpy as np

import concourse.bass as bass
import concourse.mybir as mybir
from concourse.bass_utils import run_bass_kernel_spmd

B = 256          # full batch
J = 50176        # flattened feature dim (14*14*256)
NCORES = 8
BPC = B // NCORES          # 32 samples per core
P = 128                    # SBUF partitions
RPS = P // BPC             # 4 partition-rows per sample
F = J // RPS               # 12544 free elems per partition
NCHUNK = 8
CHUNK = F // NCHUNK        # 3136 (-> [128, 3136] f32 tiles, 1.6 MB)

FP32 = mybir.dt.float32
AX = mybir.AxisListType.X

# Stashed by kernel() for test harnesses that want timing/trace info.
LAST_RESULTS = None


def _build_nc():
    nc = bass.Bass(monotonic_sem_count=0)
    c1 = nc.dram_tensor("conv1", [P, F], FP32, kind="ExternalInput")
    c2 = nc.dram_tensor("conv2", [P, F], FP32, kind="ExternalInput")
    bd = nc.dram_tensor("blockdiag", [P, P], FP32, kind="ExternalInput")
    out = nc.dram_tensor("out", [P, F], FP32, kind="ExternalOutput")

    with ExitStack() as ctx:
        ec = ctx.enter_context
        c1t = [ec(nc.sbuf_tensor(f"c1t{i}", [P, CHUNK], FP32)) for i in range(NCHUNK)]
        c2t = [ec(nc.sbuf_tensor(f"c2t{i}", [P, CHUNK], FP32)) for i in range(NCHUNK)]
        ot = [ec(nc.sbuf_tensor(f"ot{i}", [P, CHUNK], FP32)) for i in range(NCHUNK)]
        bdt = ec(nc.sbuf_tensor("bdt", [P, P], FP32))
        partials = ec(nc.sbuf_tensor("partials", [P, NCHUNK], FP32))
        sums = ec(nc.sbuf_tensor("sums", [P, 1], FP32))
        pscale = ec(nc.psum_tensor("pscale", [P, 1], FP32))

        bds = ec(nc.semaphore("bds"))
        c1s = [ec(nc.semaphore(f"c1s{i}")) for i in range(NCHUNK)]
        c2s = [ec(nc.semaphore(f"c2s{i}")) for i in range(NCHUNK)]
        c1red = ec(nc.semaphore("c1red"))
        red = ec(nc.semaphore("red"))
        mms = ec(nc.semaphore("mms"))
        muls = ec(nc.semaphore("muls"))
        sts = ec(nc.semaphore("sts"))

        # No nc.Block: instructions are emitted straight into the main basic
        # block (each tagged with its engine), which skips the Block entry
        # branches and the exit all-engine barrier.  Ring warmup: the first
        # transfer on a DGE ring runs ~2x slow, so the ACT ring warms on the
        # tiny blockdiag load and then carries c1 chunk 0 while the SP ring
        # absorbs its warmup on c1 chunk 1.
        nc.scalar.dma_start(bdt[:], bd[:]).then_inc(bds, 16)
        nc.scalar.dma_start(c1t[0][:], c1[:, bass.ts(0, CHUNK)]).then_inc(c1s[0], 16)
        for i in range(1, NCHUNK):
            nc.sync.dma_start(c1t[i][:], c1[:, bass.ts(i, CHUNK)]).then_inc(c1s[i], 16)
        for i in range(NCHUNK):
            nc.sync.dma_start(c2t[i][:], c2[:, bass.ts(i, CHUNK)]).then_inc(c2s[i], 16)

        for i in range(NCHUNK):
            nc.vector.wait_ge(c1s[i], 16)
            nc.vector.reduce_sum(
                partials[:, i : i + 1], c1t[i][:], axis=AX
            ).then_inc(c1red, 1)
        nc.vector.wait_ge(c1red, NCHUNK)
        nc.vector.reduce_sum(sums[:], partials[:], axis=AX).then_inc(red, 1)

        nc.tensor.wait_ge(bds, 16)
        nc.tensor.wait_ge(red, 1)
        nc.tensor.matmul(
            pscale[:], bdt[:], sums[:], start=True, stop=True
        ).then_inc(mms, 1)

        nc.vector.wait_ge(mms, 1)
        for i in range(NCHUNK):
            nc.vector.wait_ge(c2s[i], 16)
            nc.vector.tensor_scalar_mul(
                ot[i][:], c2t[i][:], pscale[:, 0:1]
            ).then_inc(muls, 1)

        for i in range(NCHUNK):
            nc.scalar.wait_ge(muls, i + 1)
            nc.scalar.dma_start(out[:, bass.ts(i, CHUNK)], ot[i][:]).then_inc(
                sts, 16
            )
        nc.scalar.wait_ge(sts, 16 * NCHUNK)

    return nc


def kernel(conv1, conv2, _trace=False):
    global LAST_RESULTS
    conv1 = np.ascontiguousarray(np.asarray(conv1, dtype=np.float32))
    conv2 = np.ascontiguousarray(np.asarray(conv2, dtype=np.float32))
    c1 = conv1.reshape(B, J)
    c2 = conv2.reshape(B, J)

    # blockdiag[p, m] = 1/J if p//RPS == m//RPS else 0
    bd = (
        np.kron(np.eye(BPC, dtype=np.float32), np.ones((RPS, RPS), dtype=np.float32))
        / np.float32(J)
    ).astype(np.float32)

    in_maps = []
    for i in range(NCORES):
        sl = slice(i * BPC, (i + 1) * BPC)
        in_maps.append(
            {
                "conv1": c1[sl].reshape(P, F),
                "conv2": c2[sl].reshape(P, F),
                "blockdiag": bd,
            }
        )

    nc = _build_nc()
    res = run_bass_kernel_spmd(nc, in_maps, list(range(NCORES)), trace=bool(_trace))
    LAST_RESULTS = res
    out = np.concatenate(
        [res.results[i]["out"].reshape(BPC, J) for i in range(NCORES)], axis=0
    )
    return out


# revision 19
# speedup vs baseline: 1.0696x; 1.0479x over previous
"""BilinearPooling kernel for TRN2 (8 NeuronCores, pure data parallel).

Reference math: out[b, k] = mean_j(conv1[b, j]) * conv2[b, k], with
conv1/conv2 flattened to [B, 50176] from [256, 14, 14, 256].

Sharding: batch dim B=256 split across 8 cores -> 32 samples/core.
Per-core layout: the [32, 50176] slice is viewed as [128, 12544] so sample b
occupies partitions 4b..4b+3.  A free-axis reduce gives per-partition partial
sums; one fp32 matmul against a block-diagonal (1/J) matrix sums each group of
4 partitions and broadcasts the per-sample mean back to its 4 partitions.
conv2 streams through SBUF with a per-partition scalar multiply.

Raw Bass (no Tile): the DGE DMA instruction supports at most one attached
sync-wait, so all waits are standalone engine wait_ge instructions and every
dma_start carries none.  Engine roles: SP streams the c1/c2 loads (HWDGE),
DVE does reduces + multiplies (reading the scale vector straight from PSUM),
PE does the tiny block-diag matmul, ACT loads the block-diag constant and
issues the stores (HWDGE).
"""

from contextlib import ExitStack

import numpy as np

import concourse.bass as bass
import concourse.mybir as mybir
from concourse.bass_utils import run_bass_kernel_spmd

B = 256          # full batch
J = 50176        # flattened feature dim (14*14*256)
NCORES = 8
BPC = B // NCORES          # 32 samples per core
P = 128                    # SBUF partitions
RPS = P // BPC             # 4 partition-rows per sample
F = J // RPS               # 12544 free elems per partition
NCHUNK = 8
CHUNK = F // NCHUNK        # 3136 (-> [128, 3136] f32 tiles, 1.6 MB)

FP32 = mybir.dt.float32
AX = mybir.AxisListType.X

# Stashed by kernel() for test harnesses that want timing/trace info.
LAST_RESULTS = None


def _build_nc():
    nc = bass.Bass(monotonic_sem_count=0)
    c1 = nc.dram_tensor("conv1", [P, F], FP32, kind="ExternalInput")
    c2 = nc.dram_tensor("conv2", [P, F], FP32, kind="ExternalInput")
    bd = nc.dram_tensor("blockdiag", [P, P], FP32, kind="ExternalInput")
    out = nc.dram_tensor("out", [P, F], FP32, kind="ExternalOutput")

    with ExitStack() as ctx:
        ec = ctx.enter_context
        c1t = [ec(nc.sbuf_tensor(f"c1t{i}", [P, CHUNK], FP32)) for i in range(NCHUNK)]
        c2t = [ec(nc.sbuf_tensor(f"c2t{i}", [P, CHUNK], FP32)) for i in range(NCHUNK)]
        ot = [ec(nc.sbuf_tensor(f"ot{i}", [P, CHUNK], FP32)) for i in range(NCHUNK)]
        bdt = ec(nc.sbuf_tensor("bdt", [P, P], FP32))
        partials = ec(nc.sbuf_tensor("partials", [P, NCHUNK], FP32))
        sums = ec(nc.sbuf_tensor("sums", [P, 1], FP32))
        pscale = ec(nc.psum_tensor("pscale", [P, 1], FP32))

        bds = ec(nc.semaphore("bds"))
        c1s = [ec(nc.semaphore(f"c1s{i}")) for i in range(NCHUNK)]
        c2s = [ec(nc.semaphore(f"c2s{i}")) for i in range(NCHUNK)]
        c1red = ec(nc.semaphore("c1red"))
        red = ec(nc.semaphore("red"))
        mms = ec(nc.semaphore("mms"))
        muls = ec(nc.semaphore("muls"))
        sts = ec(nc.semaphore("sts"))

        # No nc.Block: instructions are emitted straight into the main basic
        # block (each tagged with its engine), which skips the Block entry
        # branches and the exit all-engine barrier.  Ring warmup: the first
        # transfer on a DGE ring runs ~2x slow, so the ACT ring warms on the
        # tiny blockdiag load and then carries c1 chunk 0 while the SP ring
        # absorbs its warmup on c1 chunk 1.
        nc.scalar.dma_start(bdt[:], bd[:]).then_inc(bds, 16)
        for i in range(NCHUNK):
            nc.sync.dma_start(c1t[i][:], c1[:, bass.ts(i, CHUNK)]).then_inc(c1s[i], 16)
        for i in range(NCHUNK):
            nc.sync.dma_start(c2t[i][:], c2[:, bass.ts(i, CHUNK)]).then_inc(c2s[i], 16)

        for i in range(NCHUNK):
            nc.vector.wait_ge(c1s[i], 16)
            nc.vector.reduce_sum(
                partials[:, i : i + 1], c1t[i][:], axis=AX
            ).then_inc(c1red, 1)
        nc.vector.wait_ge(c1red, NCHUNK)
        nc.vector.reduce_sum(sums[:], partials[:], axis=AX).then_inc(red, 1)

        nc.tensor.wait_ge(bds, 16)
        nc.tensor.wait_ge(red, 1)
        nc.tensor.matmul(
            pscale[:], bdt[:], sums[:], start=True, stop=True
        ).then_inc(mms, 1)

        nc.vector.wait_ge(mms, 1)
        for i in range(NCHUNK):
            nc.vector.wait_ge(c2s[i], 16)
            nc.vector.tensor_scalar_mul(
                ot[i][:], c2t[i][:], pscale[:, 0:1]
            ).then_inc(muls, 1)

        for i in range(NCHUNK):
            nc.scalar.wait_ge(muls, i + 1)
            nc.scalar.dma_start(out[:, bass.ts(i, CHUNK)], ot[i][:]).then_inc(
                sts, 16
            )
        nc.scalar.wait_ge(sts, 16 * NCHUNK)

    return nc


def kernel(conv1, conv2, _trace=False):
    global LAST_RESULTS
    conv1 = np.ascontiguousarray(np.asarray(conv1, dtype=np.float32))
    conv2 = np.ascontiguousarray(np.asarray(conv2, dtype=np.float32))
    c1 = conv1.reshape(B, J)
    c2 = conv2.reshape(B, J)

    # blockdiag[p, m] = 1/J if p//RPS == m//RPS else 0
    bd = (
        np.kron(np.eye(BPC, dtype=np.float32), np.ones((RPS, RPS), dtype=np.float32))
        / np.float32(J)
    ).astype(np.float32)

    in_maps = []
    for i in range(NCORES):
        sl = slice(i * BPC, (i + 1) * BPC)
        in_maps.append(
            {
                "conv1": c1[sl].reshape(P, F),
                "conv2": c2[sl].reshape(P, F),
                "blockdiag": bd,
            }
        )

    nc = _build_nc()
    res = run_bass_kernel_spmd(nc, in_maps, list(range(NCORES)), trace=bool(_trace))
    LAST_RESULTS = res
    out = np.concatenate(
        [res.results[i]["out"].reshape(BPC, J) for i in range(NCORES)], axis=0
    )
    return out


# revision 20
# speedup vs baseline: 1.0757x; 1.0058x over previous
"""BilinearPooling kernel for TRN2 (8 NeuronCores, pure data parallel).

Reference math: out[b, k] = mean_j(conv1[b, j]) * conv2[b, k], with
conv1/conv2 flattened to [B, 50176] from [256, 14, 14, 256].

Sharding: batch dim B=256 split across 8 cores -> 32 samples/core.
Per-core layout: the [32, 50176] slice is viewed as [128, 12544] so sample b
occupies partitions 4b..4b+3.  A free-axis reduce gives per-partition partial
sums; one fp32 matmul against a block-diagonal (1/J) matrix sums each group of
4 partitions and broadcasts the per-sample mean back to its 4 partitions.
conv2 streams through SBUF with a per-partition scalar multiply.

Raw Bass (no Tile): the DGE DMA instruction supports at most one attached
sync-wait, so all waits are standalone engine wait_ge instructions and every
dma_start carries none.  Engine roles: SP streams the c1/c2 loads (HWDGE),
DVE does reduces + multiplies (reading the scale vector straight from PSUM),
PE does the tiny block-diag matmul, ACT loads the block-diag constant and
issues the stores (HWDGE).
"""

from contextlib import ExitStack

import numpy as np

import concourse.bass as bass
import concourse.mybir as mybir
from concourse.bass_utils import run_bass_kernel_spmd

B = 256          # full batch
J = 50176        # flattened feature dim (14*14*256)
NCORES = 8
BPC = B // NCORES          # 32 samples per core
P = 128                    # SBUF partitions
RPS = P // BPC             # 4 partition-rows per sample
F = J // RPS               # 12544 free elems per partition
NCHUNK = 8
CHUNK = F // NCHUNK        # 3136 (-> [128, 3136] f32 tiles, 1.6 MB)

FP32 = mybir.dt.float32
AX = mybir.AxisListType.X

# Stashed by kernel() for test harnesses that want timing/trace info.
LAST_RESULTS = None


def _build_nc():
    nc = bass.Bass(monotonic_sem_count=0)
    c1 = nc.dram_tensor("conv1", [P, F], FP32, kind="ExternalInput")
    c2 = nc.dram_tensor("conv2", [P, F], FP32, kind="ExternalInput")
    bd = nc.dram_tensor("blockdiag", [P, P], FP32, kind="ExternalInput")
    out = nc.dram_tensor("out", [P, F], FP32, kind="ExternalOutput")

    with ExitStack() as ctx:
        ec = ctx.enter_context
        c1t = [ec(nc.sbuf_tensor(f"c1t{i}", [P, CHUNK], FP32)) for i in range(NCHUNK)]
        c2t = [ec(nc.sbuf_tensor(f"c2t{i}", [P, CHUNK], FP32)) for i in range(NCHUNK)]
        ot = [ec(nc.sbuf_tensor(f"ot{i}", [P, CHUNK], FP32)) for i in range(NCHUNK)]
        bdt = ec(nc.sbuf_tensor("bdt", [P, P], FP32))
        partials = ec(nc.sbuf_tensor("partials", [P, NCHUNK], FP32))
        sums = ec(nc.sbuf_tensor("sums", [P, 1], FP32))
        pscale = ec(nc.psum_tensor("pscale", [P, 1], FP32))

        bds = ec(nc.semaphore("bds"))
        c1s = [ec(nc.semaphore(f"c1s{i}")) for i in range(NCHUNK)]
        c2s = [ec(nc.semaphore(f"c2s{i}")) for i in range(NCHUNK)]
        c1red = ec(nc.semaphore("c1red"))
        red = ec(nc.semaphore("red"))
        mms = ec(nc.semaphore("mms"))
        muls = ec(nc.semaphore("muls"))
        sts = ec(nc.semaphore("sts"))

        # No nc.Block: instructions are emitted straight into the main basic
        # block (each tagged with its engine), which skips the Block entry
        # branches and the exit all-engine barrier.  Ring warmup: the first
        # transfer on a DGE ring runs ~2x slow, so the ACT ring warms on the
        # tiny blockdiag load and then carries c1 chunk 0 while the SP ring
        # absorbs its warmup on c1 chunk 1.
        nc.scalar.dma_start(bdt[:], bd[:]).then_inc(bds, 16)
        for i in range(NCHUNK):
            nc.sync.dma_start(c1t[i][:], c1[:, bass.ts(i, CHUNK)]).then_inc(c1s[i], 16)
        for i in range(NCHUNK):
            nc.sync.dma_start(c2t[i][:], c2[:, bass.ts(i, CHUNK)]).then_inc(c2s[i], 16)

        for i in range(NCHUNK):
            nc.vector.wait_ge(c1s[i], 16)
            nc.vector.reduce_sum(
                partials[:, i : i + 1], c1t[i][:], axis=AX
            ).then_inc(c1red, 1)
        nc.vector.wait_ge(c1red, NCHUNK)
        nc.vector.reduce_sum(sums[:], partials[:], axis=AX).then_inc(red, 1)

        nc.tensor.wait_ge(bds, 16)
        nc.tensor.wait_ge(red, 1)
        nc.tensor.matmul(
            pscale[:], bdt[:], sums[:], start=True, stop=True
        ).then_inc(mms, 1)

        nc.vector.wait_ge(mms, 1)
        for i in range(NCHUNK):
            nc.vector.wait_ge(c2s[i], 16)
            nc.vector.tensor_scalar_mul(
                ot[i][:], c2t[i][:], pscale[:, 0:1]
            ).then_inc(muls, 1)

        for i in range(NCHUNK):
            # Single attached wait (the DGE ISA limit) instead of a standalone
            # engine wait: the ACT sequencer dispatches all store triggers
            # ahead of time and the ring gates each on its mul's semaphore.
            nc.scalar.dma_start(out[:, bass.ts(i, CHUNK)], ot[i][:])._wait_ge(
                muls, i + 1
            ).then_inc(sts, 16)
        nc.scalar.wait_ge(sts, 16 * NCHUNK)

    return nc


def kernel(conv1, conv2, _trace=False):
    global LAST_RESULTS
    conv1 = np.ascontiguousarray(np.asarray(conv1, dtype=np.float32))
    conv2 = np.ascontiguousarray(np.asarray(conv2, dtype=np.float32))
    c1 = conv1.reshape(B, J)
    c2 = conv2.reshape(B, J)

    # blockdiag[p, m] = 1/J if p//RPS == m//RPS else 0
    bd = (
        np.kron(np.eye(BPC, dtype=np.float32), np.ones((RPS, RPS), dtype=np.float32))
        / np.float32(J)
    ).astype(np.float32)

    in_maps = []
    for i in range(NCORES):
        sl = slice(i * BPC, (i + 1) * BPC)
        in_maps.append(
            {
                "conv1": c1[sl].reshape(P, F),
                "conv2": c2[sl].reshape(P, F),
                "blockdiag": bd,
            }
        )

    nc = _build_nc()
    res = run_bass_kernel_spmd(nc, in_maps, list(range(NCORES)), trace=bool(_trace))
    LAST_RESULTS = res
    out = np.concatenate(
        [res.results[i]["out"].reshape(BPC, J) for i in range(NCORES)], axis=0
    )
    return out


# revision 21
# speedup vs baseline: 1.1460x; 1.0653x over previous
"""BilinearPooling kernel for TRN2 (8 NeuronCores, pure data parallel).

Reference math: out[b, k] = mean_j(conv1[b, j]) * conv2[b, k], with
conv1/conv2 flattened to [B, 50176] from [256, 14, 14, 256].

Sharding: batch dim B=256 split across 8 cores -> 32 samples/core.
Per-core layout: the [32, 50176] slice is viewed as [128, 12544] so sample b
occupies partitions 4b..4b+3.  A free-axis reduce gives per-partition partial
sums; one fp32 matmul against a block-diagonal (1/J) matrix sums each group of
4 partitions and broadcasts the per-sample mean back to its 4 partitions.
conv2 streams through SBUF with a per-partition scalar multiply.

Raw Bass (no Tile): the DGE DMA instruction supports at most one attached
sync-wait, so all waits are standalone engine wait_ge instructions and every
dma_start carries none.  Engine roles: SP streams the c1/c2 loads (HWDGE),
DVE does reduces + multiplies (reading the scale vector straight from PSUM),
PE does the tiny block-diag matmul, ACT loads the block-diag constant and
issues the stores (HWDGE).
"""

from contextlib import ExitStack

import numpy as np

import concourse.bass as bass
import concourse.mybir as mybir
from concourse.bass_utils import run_bass_kernel_spmd

B = 256          # full batch
J = 50176        # flattened feature dim (14*14*256)
NCORES = 8
BPC = B // NCORES          # 32 samples per core
P = 128                    # SBUF partitions
RPS = P // BPC             # 4 partition-rows per sample
F = J // RPS               # 12544 free elems per partition
NCHUNK = 8
CHUNK = F // NCHUNK        # 3136 (-> [128, 3136] f32 tiles, 1.6 MB)

FP32 = mybir.dt.float32
AX = mybir.AxisListType.X

# Stashed by kernel() for test harnesses that want timing/trace info.
LAST_RESULTS = None


def _build_nc():
    nc = bass.Bass(monotonic_sem_count=0)
    c1 = nc.dram_tensor("conv1", [P, F], FP32, kind="ExternalInput")
    c2 = nc.dram_tensor("conv2", [P, F], FP32, kind="ExternalInput")
    bd = nc.dram_tensor("blockdiag", [P, P], FP32, kind="ExternalInput")
    out = nc.dram_tensor("out", [P, F], FP32, kind="ExternalOutput")

    with ExitStack() as ctx:
        ec = ctx.enter_context
        c1t = [ec(nc.sbuf_tensor(f"c1t{i}", [P, CHUNK], FP32)) for i in range(NCHUNK)]
        c2t = [ec(nc.sbuf_tensor(f"c2t{i}", [P, CHUNK], FP32)) for i in range(NCHUNK)]
        ot = [ec(nc.sbuf_tensor(f"ot{i}", [P, CHUNK], FP32)) for i in range(NCHUNK)]
        bdt = ec(nc.sbuf_tensor("bdt", [P, P], FP32))
        partials = ec(nc.sbuf_tensor("partials", [P, NCHUNK], FP32))
        sums = ec(nc.sbuf_tensor("sums", [P, 1], FP32))
        pscale = ec(nc.psum_tensor("pscale", [P, 1], FP32))

        bds = ec(nc.semaphore("bds"))
        c1s = [ec(nc.semaphore(f"c1s{i}")) for i in range(NCHUNK)]
        c2s = [ec(nc.semaphore(f"c2s{i}")) for i in range(NCHUNK)]
        c1red = ec(nc.semaphore("c1red"))
        red = ec(nc.semaphore("red"))
        mms = ec(nc.semaphore("mms"))
        muls = ec(nc.semaphore("muls"))
        sts = ec(nc.semaphore("sts"))

        # No nc.Block: instructions are emitted straight into the main basic
        # block (each tagged with its engine), which skips the Block entry
        # branches and the exit all-engine barrier.  Ring warmup: the first
        # transfer on a DGE ring runs ~2x slow, so the ACT ring warms on the
        # tiny blockdiag load and then carries c1 chunk 0 while the SP ring
        # absorbs its warmup on c1 chunk 1.
        nc.scalar.dma_start(bdt[:], bd[:]).then_inc(bds, 16)
        for i in range(NCHUNK):
            nc.sync.dma_start(c1t[i][:], c1[:, bass.ts(i, CHUNK)]).then_inc(c1s[i], 16)
        for i in range(NCHUNK):
            nc.sync.dma_start(c2t[i][:], c2[:, bass.ts(i, CHUNK)]).then_inc(c2s[i], 16)

        for i in range(NCHUNK):
            nc.vector.wait_ge(c1s[i], 16)
            nc.vector.reduce_sum(
                partials[:, i : i + 1], c1t[i][:], axis=AX
            ).then_inc(c1red, 1)
        nc.vector.wait_ge(c1red, NCHUNK)
        nc.vector.reduce_sum(sums[:], partials[:], axis=AX).then_inc(red, 1)

        nc.tensor.wait_ge(bds, 16)
        nc.tensor.wait_ge(red, 1)
        nc.tensor.matmul(
            pscale[:], bdt[:], sums[:], start=True, stop=True
        ).then_inc(mms, 1)

        nc.vector.wait_ge(mms, 1)
        for i in range(NCHUNK):
            nc.vector.wait_ge(c2s[i], 16)
            nc.vector.tensor_scalar_mul(
                ot[i][:], c2t[i][:], pscale[:, 0:1]
            ).then_inc(muls, 1)

        for i in range(NCHUNK):
            # Single attached wait (the DGE ISA limit) instead of a standalone
            # engine wait: the ACT sequencer dispatches all store triggers
            # ahead of time and the ring gates each on its mul's semaphore.
            nc.scalar.dma_start(out[:, bass.ts(i, CHUNK)], ot[i][:])._wait_ge(
                muls, i + 1
            ).then_inc(sts, 16)
        # EXPERIMENT: rely on the runtime epilogue for store-queue quiescence.
        # nc.scalar.wait_ge(sts, 16 * NCHUNK)

    return nc


def kernel(conv1, conv2, _trace=False):
    global LAST_RESULTS
    conv1 = np.ascontiguousarray(np.asarray(conv1, dtype=np.float32))
    conv2 = np.ascontiguousarray(np.asarray(conv2, dtype=np.float32))
    c1 = conv1.reshape(B, J)
    c2 = conv2.reshape(B, J)

    # blockdiag[p, m] = 1/J if p//RPS == m//RPS else 0
    bd = (
        np.kron(np.eye(BPC, dtype=np.float32), np.ones((RPS, RPS), dtype=np.float32))
        / np.float32(J)
    ).astype(np.float32)

    in_maps = []
    for i in range(NCORES):
        sl = slice(i * BPC, (i + 1) * BPC)
        in_maps.append(
            {
                "conv1": c1[sl].reshape(P, F),
                "conv2": c2[sl].reshape(P, F),
                "blockdiag": bd,
            }
        )

    nc = _build_nc()
    res = run_bass_kernel_spmd(nc, in_maps, list(range(NCORES)), trace=bool(_trace))
    LAST_RESULTS = res
    out = np.concatenate(
        [res.results[i]["out"].reshape(BPC, J) for i in range(NCORES)], axis=0
    )
    return out
